# revision 10
# baseline (speedup 1.0000x reference)
"""CBOW (one-hot embedding lookup + mean + output matmul + softmax) on 8
Trainium2 NeuronCores, vocab-sharded end to end.

Full problem: batch [1024, 10, 32000] f32 one-hot, emb [32000, 128] f32,
w_out [128, 32000] f32 -> softmax(mean_c(batch @ emb) @ w_out) [1024, 32000].

Sharding: core i owns vocab columns [i*4000, (i+1)*4000). It receives
  batch_s [1024, 10, 4000] f32  (full batch, its vocab slice)
  emb_s   [4096, 128]      f32  (its emb rows, zero-padded 4000->4096)
  w_out_s [128, 4000]      f32  (its output-projection columns)
and produces out_s [1024, 4000] bf16 (its softmax columns; host concatenates
along vocab and upcasts to f32).

Batch rows run in 8 blocks of 128. Stage 1 streams each block's one-hot
slice through a casting DMA (f32 DRAM -> bf16 SBUF); per 128-wide v-tile
the 10 context planes are summed on the PE as REGULAR bf16 matmuls
(lhsT=oh_c, rhs=identity accumulates oh_c.T in fp32 PSUM) giving sT[v, b],
then sumT_bb[d, b] += emb_tile.T @ sT over the core's 32 v-tiles.

Cross-core reduction is batched into 3 waves of blocks (0-3, 4-5, 6-7) so
only 6 collectives run in total. A collective trigger BLOCKS the issuing
gpsimd queue for the collective's full duration, and gpsimd is also the
only engine that can issue the casting one-hot DMAs - so each trigger is
scheduled (via an explicit event table keyed on (block, chunk)) at a point
where its input is already available and >=3 one-hot chunks are queued
ahead in the DMA rings, letting the SDMA engines keep streaming while
gpsimd waits. Stage 2 (logits matmul; scalar-engine exp(x/C) with fused
row-sum via accum_out; tiny denominator AllReduce per wave; per-partition
scale; bf16 DMA out) is interleaved into later blocks' streaming windows.

Engine roles: gpsimd = one-hot casting DMAs + collective triggers;
sync = producer-side DMAs only (weights, bounce-buffer writes - never
waits on a collective); scalar = collective-output reads, exp/sums/scales,
output writes (all waits naturally in its dependency chain); vector = PSUM
copies + reciprocals only. PE never sees a matmul whose inputs aren't
already resident.
"""

from contextlib import ExitStack

import numpy as np

import concourse.bass as bass
import concourse.tile as tile
from concourse import bacc, masks, mybir
from concourse._compat import with_exitstack

F32 = mybir.dt.float32
BF16 = mybir.dt.bfloat16
AX = mybir.AxisListType
AF = mybir.ActivationFunctionType

B_FULL, C, V, D = 1024, 10, 32000, 128
N_CORES = 8
VS = V // N_CORES          # 4000 vocab columns per core
VS_PAD = 4096              # emb rows padded to a multiple of 128
N_TILES = VS_PAD // 128    # 32 v-tiles (last is 32 valid rows)
BB = 128                   # batch rows per block
N_BB = B_FULL // BB        # 8 blocks
VC = 1024                  # one-hot v-chunk (chunks: 1024,1024,1024,928)
NC2 = 512                  # stage-2 logits chunk

WAVES = [(0, 3), (3, 5), (5, 7), (7, 8)]


@with_exitstack
def _cbow_kernel(ctx: ExitStack, tc, out, batch, emb, w_out):
    nc = tc.nc
    Bs, Cs, Vs = batch.shape
    assert Bs == B_FULL and Cs == C and Vs == VS
    rg = [list(range(N_CORES))]
    n_vc = (Vs + VC - 1) // VC
    n_nc = (Vs + NC2 - 1) // NC2

    const_pool = ctx.enter_context(tc.tile_pool(name="const", bufs=1))
    ident = const_pool.tile([128, 128], BF16)
    masks.make_identity(nc, ident[:])

    eb_pool = ctx.enter_context(tc.tile_pool(name="eb", bufs=1))
    eb = eb_pool.tile([128, N_TILES, 128], F32)
    nc.sync.dma_start(eb[:], emb.rearrange("(n p) d -> p n d", p=128))
    wo_pool = ctx.enter_context(tc.tile_pool(name="wo", bufs=1))
    wo = wo_pool.tile([128, VS], F32)
    nc.sync.dma_start(wo[:], w_out)

    oh_pool = ctx.enter_context(tc.tile_pool(name="oh", bufs=4))
    sT_pool = ctx.enter_context(tc.tile_pool(name="sT", bufs=4))
    sTps_pool = ctx.enter_context(tc.tile_pool(name="sTps", bufs=3, space="PSUM"))
    acc_pool = ctx.enter_context(tc.tile_pool(name="acc", bufs=2, space="PSUM"))
    avgsb_pool = ctx.enter_context(tc.tile_pool(name="avgsb", bufs=2))
    avgg_pool = ctx.enter_context(tc.tile_pool(name="avgg", bufs=2))
    lg_pool = ctx.enter_context(tc.tile_pool(name="lg", bufs=5))
    lgps_pool = ctx.enter_context(tc.tile_pool(name="lgps", bufs=3, space="PSUM"))
    stat_pool = ctx.enter_context(tc.tile_pool(name="stat", bufs=2))
    dram = ctx.enter_context(tc.tile_pool(name="dram", bufs=3, space="DRAM"))

    # wave index -> state dict
    wstate = {}
    for wi, (w0, w1) in enumerate(WAVES):
        cols = (w1 - w0) * BB
        wstate[wi] = {
            "w0": w0,
            "w1": w1,
            "avg_sb": avgsb_pool.tile([128, cols], F32, tag=f"avgsb{wi}", name=f"avgsb{wi}"),
            "den_sb": stat_pool.tile([128, w1 - w0], F32, tag=f"densb{wi}", name=f"densb{wi}"),
            "lg": {},
        }

    bb2wave = {}
    for wi, (w0, w1) in enumerate(WAVES):
        for bb in range(w0, w1):
            bb2wave[bb] = wi

    def stage1_chunk(bb, j, avgT_ps):
        b0 = bb * BB
        v0 = j * VC
        vc = min(VC, Vs - v0)
        oh = oh_pool.tile([128, Cs, VC], BF16, tag="oh")
        nc.gpsimd.dma_start(oh[:, :, :vc], batch[b0 : b0 + BB, :, v0 : v0 + vc])
        nt = (vc + 127) // 128
        for t in range(nt):
            toff = t * 128
            tw = min(128, vc - toff)
            g = j * (VC // 128) + t
            sT_ps = sTps_pool.tile([128, BB], F32, tag="sTps")
            for c in range(Cs):
                nc.tensor.matmul(
                    sT_ps[:tw],
                    lhsT=oh[:, c, toff : toff + tw],
                    rhs=ident[:],
                    start=(c == 0),
                    stop=(c == Cs - 1),
                )
            sT = sT_pool.tile([128, BB], F32, tag="sT")
            nc.vector.tensor_copy(sT[:tw], sT_ps[:tw])
            nc.tensor.matmul(
                avgT_ps[:],
                lhsT=eb[:tw, g, :],
                rhs=sT[:tw],
                start=(g == 0),
                stop=(g == N_TILES - 1),
            )

    def avg_ar(wi):
        """Bounce the wave's context-sums to DRAM, AllReduce, read back."""
        s = wstate[wi]
        cols = (s["w1"] - s["w0"]) * BB
        cc_in = dram.tile([128, cols], F32, tag=f"cc_in{wi}")
        cc_out = dram.tile(
            [128, cols], F32, tag=f"cc_out{wi}", addr_space="Shared"
        )
        nc.sync.dma_start(cc_in[:], s["avg_sb"][:])
        nc.gpsimd.collective_compute(
            "AllReduce",
            mybir.AluOpType.add,
            replica_groups=rg,
            ins=[cc_in.opt()],
            outs=[cc_out.opt()],
        )
        avg_g = avgg_pool.tile([128, cols], F32, tag=f"avgg{wi}")
        nc.scalar.dma_start(avg_g[:], cc_out[:])
        s["avg_g"] = avg_g

    def den_ar(wi):
        s = wstate[wi]
        nb = s["w1"] - s["w0"]
        cc_in = dram.tile([128, nb], F32, tag=f"cc2_in{wi}")
        cc_out = dram.tile(
            [128, nb], F32, tag=f"cc2_out{wi}", addr_space="Shared"
        )
        nc.sync.dma_start(cc_in[:], s["den_sb"][:])
        nc.gpsimd.collective_compute(
            "AllReduce",
            mybir.AluOpType.add,
            replica_groups=rg,
            ins=[cc_in.opt()],
            outs=[cc_out.opt()],
        )
        s["cc2_out"] = cc_out

    def stage2a(bb):
        """Logits + exp (fused row-sum) + this block's local denominator."""
        wi = bb2wave[bb]
        s = wstate[wi]
        slot = bb - s["w0"]
        avg_g = s["avg_g"]
        lg = lg_pool.tile([128, VS], BF16, tag="lg")
        sums = stat_pool.tile([128, n_nc], F32, tag="sums")
        for k in range(n_nc):
            n0 = k * NC2
            nw = min(NC2, Vs - n0)
            lg_ps = lgps_pool.tile([128, NC2], F32, tag="lgps")
            nc.tensor.matmul(
                lg_ps[:, :nw],
                lhsT=avg_g[:, slot * BB : (slot + 1) * BB],
                rhs=wo[:, n0 : n0 + nw],
                start=True,
                stop=True,
            )
            # exp(x / C): the 1/C mean fold; fused row-sum via accum_out
            nc.scalar.activation(
                lg[:, n0 : n0 + nw],
                lg_ps[:, :nw],
                AF.Exp,
                scale=1.0 / Cs,
                accum_out=sums[:, k : k + 1],
            )
        scr = stat_pool.tile([128, n_nc], F32, tag="scr")
        nc.scalar.activation(
            scr[:, :n_nc],
            sums[:, :n_nc],
            AF.Copy,
            accum_out=s["den_sb"][:, slot : slot + 1],
        )
        s["lg"][bb] = lg

    def stage2b(wi):
        """Global denominator -> reciprocal -> scale -> bf16 out."""
        s = wstate[wi]
        nb = s["w1"] - s["w0"]
        den_g = stat_pool.tile([128, nb], F32, tag=f"deng{wi}")
        nc.scalar.dma_start(den_g[:], s["cc2_out"][:])
        r = stat_pool.tile([128, nb], F32, tag=f"recip{wi}")
        nc.vector.reciprocal(r[:], den_g[:])
        for bb in range(s["w0"], s["w1"]):
            slot = bb - s["w0"]
            lg = s["lg"].pop(bb)
            for k in range(n_nc):
                n0 = k * NC2
                nw = min(NC2, Vs - n0)
                if k % 2 == 0:
                    nc.scalar.mul(
                        lg[:, n0 : n0 + nw],
                        lg[:, n0 : n0 + nw],
                        r[:, slot : slot + 1],
                    )
                else:
                    nc.vector.tensor_scalar_mul(
                        lg[:, n0 : n0 + nw],
                        lg[:, n0 : n0 + nw],
                        r[:, slot : slot + 1],
                    )
            b0 = bb * BB
            nc.scalar.dma_start(out[b0 : b0 + BB, :], lg[:])

    # event table: emit these right after stage1_chunk(bb, j). Slots are
    # placed so each collective trigger's input is ready by the time the
    # gpsimd issue front (which runs ~3 chunks ahead of data arrival)
    # reaches it, and the SDMA backlog covers the trigger's blocking wait.
    events = {
        (3, 3): [lambda: avg_ar(0)],
        (4, 0): [lambda: stage2a(0)],
        (4, 1): [lambda: stage2a(1)],
        (4, 2): [lambda: stage2a(2)],
        (5, 0): [lambda: den_ar(0)],
        (5, 2): [lambda: stage2b(0)],
        (5, 3): [lambda: avg_ar(1)],
        (6, 0): [lambda: stage2a(3)],
        (6, 1): [lambda: stage2a(4)],
        (6, 2): [lambda: den_ar(1)],
        (7, 0): [lambda: stage2b(1)],
        (7, 2): [lambda: avg_ar(2)],
        (7, 3): [lambda: stage2a(5), lambda: stage2a(6)],
    }

    for bb in range(N_BB):
        avgT_ps = acc_pool.tile([128, BB], F32, tag="acc")
        for j in range(n_vc):
            stage1_chunk(bb, j, avgT_ps)
            for fn in events.get((bb, j), []):
                fn()
        # park this block's context-sums in its wave buffer
        wi = bb2wave[bb]
        s = wstate[wi]
        slot = bb - s["w0"]
        nc.vector.tensor_copy(
            s["avg_sb"][:, slot * BB : (slot + 1) * BB], avgT_ps[:]
        )

    # tail: wave 2 epilogue overlaps the last stream arrivals; wave 3
    # (a single block) is the only fully-serial remainder.
    den_ar(2)
    stage2b(2)
    avg_ar(3)
    stage2a(7)
    den_ar(3)
    stage2b(3)


def build(num_devices=N_CORES):
    nc = bacc.Bacc(
        "TRN2",
        target_bir_lowering=False,
        debug=False,
        num_devices=num_devices,
        num_swdge_queues=4,
    )
    batch = nc.dram_tensor(
        "batch", [B_FULL, C, VS], F32, kind="ExternalInput"
    ).ap()
    emb = nc.dram_tensor("emb", [VS_PAD, D], F32, kind="ExternalInput").ap()
    w_out = nc.dram_tensor("w_out", [D, VS], F32, kind="ExternalInput").ap()
    out = nc.dram_tensor("out", [B_FULL, VS], BF16, kind="ExternalOutput").ap()
    with tile.TileContext(nc) as tc:
        _cbow_kernel(tc, out, batch, emb, w_out)
    nc.compile()
    return nc


_NC = None


def _build_cached():
    global _NC
    if _NC is None:
        _NC = build()
    return _NC


def _run(batch, emb, w_out, trace=False, **kwargs):
    from concourse.bass_utils import run_bass_kernel_spmd

    nc = _build_cached()
    batch = np.ascontiguousarray(np.asarray(batch, dtype=np.float32))
    emb = np.asarray(emb, dtype=np.float32)
    w_out = np.asarray(w_out, dtype=np.float32)
    in_maps = []
    for i in range(N_CORES):
        v0 = i * VS
        emb_pad = np.zeros((VS_PAD, D), dtype=np.float32)
        emb_pad[:VS] = emb[v0 : v0 + VS]
        in_maps.append(
            {
                "batch": np.ascontiguousarray(batch[:, :, v0 : v0 + VS]),
                "emb": emb_pad,
                "w_out": np.ascontiguousarray(w_out[:, v0 : v0 + VS]),
            }
        )
    res = run_bass_kernel_spmd(
        nc, in_maps, core_ids=list(range(N_CORES)), trace=trace, **kwargs
    )
    out = np.concatenate(
        [r["out"].astype(np.float32) for r in res.results], axis=1
    )
    return out, res


def kernel(batch, emb, w_out):
    out, _ = _run(batch, emb, w_out, trace=False)
    return out


# revision 12
# speedup vs baseline: 1.0772x; 1.0772x over previous
"""CBOW (one-hot embedding lookup + mean + output matmul + softmax) on 8
Trainium2 NeuronCores, vocab-sharded end to end.

Full problem: batch [1024, 10, 32000] f32 one-hot, emb [32000, 128] f32,
w_out [128, 32000] f32 -> softmax(mean_c(batch @ emb) @ w_out) [1024, 32000].

Sharding: core i owns vocab columns [i*4000, (i+1)*4000). It receives
  batch_s [1024, 10, 4000] f32  (full batch, its vocab slice)
  emb_s   [4096, 128]      f32  (its emb rows, zero-padded 4000->4096)
  w_out_s [128, 4000]      f32  (its output-projection columns)
and produces out_s [1024, 4000] bf16 (its softmax columns; host concatenates
along vocab and upcasts to f32).

Batch rows run in 8 blocks of 128. Stage 1 streams each block's one-hot
slice as f32 via HWDGE on the sync engine (which carries no waits that
depend on collectives, so the stream issue front never stalls), casts to
bf16 on-chip (context planes 0-4 on DVE, 5-9 on the scalar engine), then
per 128-wide v-tile sums the 10 context planes on the PE as REGULAR bf16
matmuls (lhsT=oh_c, rhs=identity accumulates oh_c.T in fp32 PSUM) giving
sT[v, b]; sumT_bb[d, b] += emb_tile.T @ sT over the core's 32 v-tiles.

Cross-core reduction runs in 4 waves of blocks (0-2, 3-4, 5-6, 7). A
collective trigger blocks its issuing engine for the collective's full
duration, so gpsimd is dedicated to collectives: triggers plus the
bounce-buffer READS (whose waits are the natural in-order chain there).
Bounce WRITES ride sync (they wait only on local compute). Stage 2
(logits matmul; scalar exp(x/C) with fused row-sum via accum_out; tiny
per-wave denominator AllReduce; per-partition scale split DVE/scalar;
bf16 DMA out) is placed in later blocks' streaming windows at slots
chosen so every consumer reaches its collective-dependent input only
after the collective has finished.
"""

from contextlib import ExitStack

import numpy as np

import concourse.bass as bass
import concourse.tile as tile
from concourse import bacc, masks, mybir
from concourse._compat import with_exitstack

F32 = mybir.dt.float32
BF16 = mybir.dt.bfloat16
AX = mybir.AxisListType
AF = mybir.ActivationFunctionType

B_FULL, C, V, D = 1024, 10, 32000, 128
N_CORES = 8
VS = V // N_CORES          # 4000 vocab columns per core
VS_PAD = 4096              # emb rows padded to a multiple of 128
N_TILES = VS_PAD // 128    # 32 v-tiles (last is 32 valid rows)
BB = 128                   # batch rows per block
N_BB = B_FULL // BB        # 8 blocks
VC = 1024                  # one-hot v-chunk (chunks: 1024,1024,1024,928)
NC2 = 512                  # stage-2 logits chunk
C_DVE = 5                  # context planes cast on DVE; rest on scalar

WAVES = [(0, 3), (3, 5), (5, 7), (7, 8)]


@with_exitstack
def _cbow_kernel(ctx: ExitStack, tc, out, batch, emb, w_out):
    nc = tc.nc
    Bs, Cs, Vs = batch.shape
    assert Bs == B_FULL and Cs == C and Vs == VS
    rg = [list(range(N_CORES))]
    n_vc = (Vs + VC - 1) // VC
    n_nc = (Vs + NC2 - 1) // NC2

    const_pool = ctx.enter_context(tc.tile_pool(name="const", bufs=1))
    ident = const_pool.tile([128, 128], BF16)
    masks.make_identity(nc, ident[:])

    eb_pool = ctx.enter_context(tc.tile_pool(name="eb", bufs=1))
    eb = eb_pool.tile([128, N_TILES, 128], F32)
    nc.sync.dma_start(eb[:], emb.rearrange("(n p) d -> p n d", p=128))
    wo_pool = ctx.enter_context(tc.tile_pool(name="wo", bufs=1))
    wo = wo_pool.tile([128, VS], F32)
    nc.sync.dma_start(wo[:], w_out)

    oh32_pool = ctx.enter_context(tc.tile_pool(name="oh32", bufs=2))
    ohb_pool = ctx.enter_context(tc.tile_pool(name="ohb", bufs=2))
    sT_pool = ctx.enter_context(tc.tile_pool(name="sT", bufs=4))
    sTps_pool = ctx.enter_context(tc.tile_pool(name="sTps", bufs=3, space="PSUM"))
    acc_pool = ctx.enter_context(tc.tile_pool(name="acc", bufs=2, space="PSUM"))
    avgsb_pool = ctx.enter_context(tc.tile_pool(name="avgsb", bufs=1))
    avgg_pool = ctx.enter_context(tc.tile_pool(name="avgg", bufs=1))
    lg_pool = ctx.enter_context(tc.tile_pool(name="lg", bufs=4))
    lgps_pool = ctx.enter_context(tc.tile_pool(name="lgps", bufs=3, space="PSUM"))
    stat_pool = ctx.enter_context(tc.tile_pool(name="stat", bufs=2))
    dram = ctx.enter_context(tc.tile_pool(name="dram", bufs=2, space="DRAM"))

    # wave index -> state dict
    wstate = {}
    for wi, (w0, w1) in enumerate(WAVES):
        cols = (w1 - w0) * BB
        wstate[wi] = {
            "w0": w0,
            "w1": w1,
            "avg_sb": avgsb_pool.tile(
                [128, cols], F32, tag=f"avgsb{wi}", name=f"avgsb{wi}"
            ),
            "den_sb": stat_pool.tile(
                [128, w1 - w0], F32, tag=f"densb{wi}", name=f"densb{wi}"
            ),
            "lg": {},
        }

    bb2wave = {}
    for wi, (w0, w1) in enumerate(WAVES):
        for bb in range(w0, w1):
            bb2wave[bb] = wi

    def stage1_chunk(bb, j, avgT_ps, dve_only_cast=False):
        b0 = bb * BB
        v0 = j * VC
        vc = min(VC, Vs - v0)
        oh32 = oh32_pool.tile([128, Cs, VC], F32, tag="oh32")
        nc.sync.dma_start(
            oh32[:, :, :vc], batch[b0 : b0 + BB, :, v0 : v0 + vc]
        )
        ohb = ohb_pool.tile([128, Cs, VC], BF16, tag="ohb")
        if dve_only_cast:
            # keep the scalar engine free for the epilogue exp/scale chain
            nc.vector.tensor_copy(ohb[:, :, :vc], oh32[:, :, :vc])
        else:
            nc.vector.tensor_copy(ohb[:, :C_DVE, :vc], oh32[:, :C_DVE, :vc])
            nc.scalar.copy(ohb[:, C_DVE:, :vc], oh32[:, C_DVE:, :vc])
        nt = (vc + 127) // 128
        for t in range(nt):
            toff = t * 128
            tw = min(128, vc - toff)
            g = j * (VC // 128) + t
            sT_ps = sTps_pool.tile([128, BB], F32, tag="sTps")
            for c in range(Cs):
                nc.tensor.matmul(
                    sT_ps[:tw],
                    lhsT=ohb[:, c, toff : toff + tw],
                    rhs=ident[:],
                    start=(c == 0),
                    stop=(c == Cs - 1),
                )
            sT = sT_pool.tile([128, BB], F32, tag="sT")
            nc.vector.tensor_copy(sT[:tw], sT_ps[:tw])
            nc.tensor.matmul(
                avgT_ps[:],
                lhsT=eb[:tw, g, :],
                rhs=sT[:tw],
                start=(g == 0),
                stop=(g == N_TILES - 1),
            )

    def avg_ar(wi):
        """Bounce the wave's context-sums to DRAM, AllReduce, read back."""
        s = wstate[wi]
        cols = (s["w1"] - s["w0"]) * BB
        cc_in = dram.tile([128, cols], F32, tag=f"cc_in{wi}", bufs=1)
        cc_out = dram.tile(
            [128, cols], F32, tag=f"cc_out{wi}", addr_space="Shared", bufs=1
        )
        nc.sync.dma_start(cc_in[:], s["avg_sb"][:])
        nc.gpsimd.collective_compute(
            "AllReduce",
            mybir.AluOpType.add,
            replica_groups=rg,
            ins=[cc_in.opt()],
            outs=[cc_out.opt()],
        )
        avg_g = avgg_pool.tile([128, cols], F32, tag=f"avgg{wi}")
        nc.gpsimd.dma_start(avg_g[:], cc_out[:])
        s["avg_g"] = avg_g

    def den_ar(wi):
        s = wstate[wi]
        nb = s["w1"] - s["w0"]
        cc_in = dram.tile([128, nb], F32, tag=f"cc2_in{wi}", bufs=1)
        cc_out = dram.tile(
            [128, nb], F32, tag=f"cc2_out{wi}", addr_space="Shared", bufs=1
        )
        nc.sync.dma_start(cc_in[:], s["den_sb"][:])
        nc.gpsimd.collective_compute(
            "AllReduce",
            mybir.AluOpType.add,
            replica_groups=rg,
            ins=[cc_in.opt()],
            outs=[cc_out.opt()],
        )
        den_g = stat_pool.tile([128, nb], F32, tag=f"deng{wi}")
        nc.gpsimd.dma_start(den_g[:], cc_out[:])
        s["den_g"] = den_g

    def stage2a(bb):
        """Logits + exp (fused row-sum) + this block's local denominator."""
        wi = bb2wave[bb]
        s = wstate[wi]
        slot = bb - s["w0"]
        avg_g = s["avg_g"]
        lg = lg_pool.tile([128, VS], BF16, tag="lg")
        sums = stat_pool.tile([128, n_nc], F32, tag="sums")
        for k in range(n_nc):
            n0 = k * NC2
            nw = min(NC2, Vs - n0)
            lg_ps = lgps_pool.tile([128, NC2], F32, tag="lgps")
            nc.tensor.matmul(
                lg_ps[:, :nw],
                lhsT=avg_g[:, slot * BB : (slot + 1) * BB],
                rhs=wo[:, n0 : n0 + nw],
                start=True,
                stop=True,
            )
            # exp(x / C): the 1/C mean fold; fused row-sum via accum_out
            nc.scalar.activation(
                lg[:, n0 : n0 + nw],
                lg_ps[:, :nw],
                AF.Exp,
                scale=1.0 / Cs,
                accum_out=sums[:, k : k + 1],
            )
        scr = stat_pool.tile([128, n_nc], F32, tag="scr")
        nc.scalar.activation(
            scr[:, :n_nc],
            sums[:, :n_nc],
            AF.Copy,
            accum_out=s["den_sb"][:, slot : slot + 1],
        )
        s["lg"][bb] = lg

    def stage2b(wi):
        """Global denominator -> reciprocal -> scale -> bf16 out."""
        s = wstate[wi]
        nb = s["w1"] - s["w0"]
        r = stat_pool.tile([128, nb], F32, tag=f"recip{wi}")
        nc.vector.reciprocal(r[:], s["den_g"][:])
        for bb in range(s["w0"], s["w1"]):
            slot = bb - s["w0"]
            lg = s["lg"].pop(bb)
            for k in range(n_nc):
                n0 = k * NC2
                nw = min(NC2, Vs - n0)
                if k % 2 == 0:
                    nc.scalar.mul(
                        lg[:, n0 : n0 + nw],
                        lg[:, n0 : n0 + nw],
                        r[:, slot : slot + 1],
                    )
                else:
                    nc.vector.tensor_scalar_mul(
                        lg[:, n0 : n0 + nw],
                        lg[:, n0 : n0 + nw],
                        r[:, slot : slot + 1],
                    )
            b0 = bb * BB
            nc.scalar.dma_start(out[b0 : b0 + BB, :], lg[:])

    # event table: emit these right after stage1_chunk(bb, j). Slots put
    # each collective-dependent consumer 2+ chunks after the collective's
    # expected completion so arrival-paced engines never head-of-line block.
    events = {
        (3, 3): [lambda: avg_ar(0)],
        (4, 1): [lambda: stage2a(0)],
        (4, 2): [lambda: stage2a(1)],
        (4, 3): [lambda: stage2a(2)],
        (5, 1): [lambda: den_ar(0)],
        (5, 3): [lambda: stage2b(0)],
        (6, 0): [lambda: avg_ar(1)],
        (6, 2): [lambda: stage2a(3)],
        (6, 3): [lambda: stage2a(4)],
        (7, 0): [lambda: den_ar(1)],
        (7, 1): [lambda: avg_ar(2), lambda: stage2a(5)],
        (7, 2): [lambda: stage2b(1), lambda: stage2a(6)],
        (7, 3): [lambda: den_ar(2)],
    }

    for bb in range(N_BB):
        avgT_ps = acc_pool.tile([128, BB], F32, tag="acc")
        for j in range(n_vc):
            stage1_chunk(bb, j, avgT_ps, dve_only_cast=(bb == N_BB - 1))
            for fn in events.get((bb, j), []):
                fn()
        # park this block's context-sums in its wave buffer
        wi = bb2wave[bb]
        s = wstate[wi]
        slot = bb - s["w0"]
        nc.vector.tensor_copy(
            s["avg_sb"][:, slot * BB : (slot + 1) * BB], avgT_ps[:]
        )

    # tail: wave 2 epilogue overlaps the last arrivals; wave 3 (one block)
    # is the only fully-serial remainder.
    stage2b(2)
    avg_ar(3)
    stage2a(7)
    den_ar(3)
    stage2b(3)


def build(num_devices=N_CORES):
    nc = bacc.Bacc(
        "TRN2",
        target_bir_lowering=False,
        debug=False,
        num_devices=num_devices,
        num_swdge_queues=4,
    )
    batch = nc.dram_tensor(
        "batch", [B_FULL, C, VS], F32, kind="ExternalInput"
    ).ap()
    emb = nc.dram_tensor("emb", [VS_PAD, D], F32, kind="ExternalInput").ap()
    w_out = nc.dram_tensor("w_out", [D, VS], F32, kind="ExternalInput").ap()
    out = nc.dram_tensor("out", [B_FULL, VS], BF16, kind="ExternalOutput").ap()
    with tile.TileContext(nc) as tc:
        _cbow_kernel(tc, out, batch, emb, w_out)
    nc.compile()
    return nc


_NC = None


def _build_cached():
    global _NC
    if _NC is None:
        _NC = build()
    return _NC


def _run(batch, emb, w_out, trace=False, **kwargs):
    from concourse.bass_utils import run_bass_kernel_spmd

    nc = _build_cached()
    batch = np.ascontiguousarray(np.asarray(batch, dtype=np.float32))
    emb = np.asarray(emb, dtype=np.float32)
    w_out = np.asarray(w_out, dtype=np.float32)
    in_maps = []
    for i in range(N_CORES):
        v0 = i * VS
        emb_pad = np.zeros((VS_PAD, D), dtype=np.float32)
        emb_pad[:VS] = emb[v0 : v0 + VS]
        in_maps.append(
            {
                "batch": np.ascontiguousarray(batch[:, :, v0 : v0 + VS]),
                "emb": emb_pad,
                "w_out": np.ascontiguousarray(w_out[:, v0 : v0 + VS]),
            }
        )
    res = run_bass_kernel_spmd(
        nc, in_maps, core_ids=list(range(N_CORES)), trace=trace, **kwargs
    )
    out = np.concatenate(
        [r["out"].astype(np.float32) for r in res.results], axis=1
    )
    return out, res


def kernel(batch, emb, w_out):
    out, _ = _run(batch, emb, w_out, trace=False)
    return out
